# revision 36
# baseline (speedup 1.0000x reference)
"""Multi-headed attention (B=2, S=2048, D=768, H=12) on 8 TRN2 NeuronCores.

Sharding: data parallel on batch x tensor parallel on heads. Core c handles
batch c//4 and heads 3*(c%4) .. 3*(c%4)+2. Each core computes its partial
output projection [S, D]; the host sums the 4 partials per batch.

Key-position compaction: the mask is per key position only ([B,1,1,S],
values 0/1). Masked keys contribute exp(-1e9) == 0.0 exactly (fp32
underflow) to every softmax row, so the host drops masked key/value
positions before projection and pads to a multiple of 128; padded rows get
a -1e9 additive bias on the scores (same underflow-to-zero as the
reference's where(mask==0, -1e9, scores)). This is exact, not approximate.

Softmax runs without max-subtraction: scores ~ N(0,1) after the 1/sqrt(dk)
scale, so exp() cannot overflow; the reference's max-subtraction only
shifts numerator and denominator by a common factor.

Differences vs the first working version (283 us -> ~160 us):
  * bf16 operands everywhere on the matmul path (halves HBM traffic and
    SBUF footprint; PE rate is 1 cycle/row for bf16 same as fp32r).
  * scores tiles are [128, 1024] PSUM pairs (two banks, one matmul per
    bank) so each ScalarE exp instruction covers 1024 columns - halves
    the per-instruction overhead on the engine that paces attention.
  * Q-projection and output-projection matmul groups are interleaved
    between attention (j,h) units so the in-order PE queue always holds
    independent work: the HAM clock gate re-throttles the PE to 1.2 GHz
    whenever it sees idle gaps, which is where most of the baseline's
    time went (231 us of 291 us at K=4/8). A thermal/power duty cap
    still forces ~50% K=4/8 after ~45 us of sustained matmul activity;
    run-to-run variance from chip temperature is ~+/-15 us.
  * softmax normalization: u (PV output, with the ones-row denominator)
    drains to SBUF immediately (denominator row via ScalarE so the
    reciprocal can start; numerators via DVE) which frees the u PSUM
    bank for the next unit ~4 us earlier; 1/denom uses the single-op
    reciprocal_approx_fast (the exact InstReciprocal costs ~6.5 us per
    [1,1024] row); the column broadcast runs on the otherwise-idle
    GpSimd engine (partition_broadcast), not a PE matmul.
    NOTE: reciprocal_approx_fast silently produces garbage when its
    input AP is in PSUM or at a nonzero partition offset - it must read
    a [1, N] SBUF tile at partition 0.
  * DMAs are ordered strictly in consumption order; the v_aug ones
    columns come from an engine memset (the strided DMA cost thousands
    of descriptors); mask-bias and biases ride in one merged 'smalls'
    tensor (each extra DMA costs ~620 ns of issue time on the queue).

On-device layouts (per core):
  qT [e_local, s]   e_local = 3 local heads x 64 = 192, stored as a
                    [128, 2048] pair tile (heads 0,1) + [64, 2048] tile
  kT [e_local, kpos] same split, kpos compacted+padded to S_pad
  v_aug [128, KB*3*65] - per (kblock, head): 64 v columns + a ones column
                    (the ones column makes the PV matmul also produce the
                    softmax denominator as row 64 of the PSUM tile)
  scores are computed transposed, sT[kpos, q], so the pad-bias is a
  per-partition scalar and exp() needs a single ScalarE pass per tile.
"""

import sys

for _p in ("/opt/trn_rl_repo",):
    if _p not in sys.path:
        sys.path.insert(0, _p)

import numpy as np
import ml_dtypes

import concourse.bacc as bacc
import concourse.mybir as mybir
import concourse.tile as tile

B, S, D, H = 2, 2048, 768, 12
DK = D // H          # 64
NH = 3               # heads per core
E = NH * DK          # 192 local e width
N_CORES = 8
QW = 1024            # attention q tile (two PSUM banks)
QC = S // QW         # 2
DCH = D // 128       # 6 contraction chunks for the projections
NEG = -1.0e9

F32 = mybir.dt.float32
F32R = mybir.dt.float32r
BF16 = mybir.dt.bfloat16
BF_NP = ml_dtypes.bfloat16


def _build_program(kb: int):
    """Build the single-core SPMD program for KB key blocks of 128."""
    sk = kb * 128
    nc = bacc.Bacc("TRN2", target_bir_lowering=False, debug=False)

    xq = nc.dram_tensor("xq_t", [D, S], BF16, kind="ExternalInput").ap()
    xk = nc.dram_tensor("xk_t", [D, sk], BF16, kind="ExternalInput").ap()
    xv = nc.dram_tensor("xv_t", [D, sk], BF16, kind="ExternalInput").ap()
    wq = nc.dram_tensor("wq_t", [128, DCH * E], BF16, kind="ExternalInput").ap()
    wk = nc.dram_tensor("wk_t", [128, DCH * E], BF16, kind="ExternalInput").ap()
    wv = nc.dram_tensor("wv_t", [128, DCH * E], BF16, kind="ExternalInput").ap()
    wo = nc.dram_tensor("wo_t", [E, D], BF16, kind="ExternalInput").ap()
    sm = nc.dram_tensor("smalls", [128, kb + 4], F32, kind="ExternalInput").ap()
    out = nc.dram_tensor("out", [S, D], BF16, kind="ExternalOutput").ap()

    exp_f = mybir.ActivationFunctionType.Exp

    with tile.TileContext(nc) as tc:
        with (
            tc.tile_pool(name="resident", bufs=1) as res,
            tc.tile_pool(name="eT", bufs=4) as etp,
            tc.tile_pool(name="small", bufs=2) as small,
            tc.tile_pool(name="ocp", bufs=6) as ocp,
            tc.tile_pool(name="big_ps", bufs=2, space="PSUM") as big,     # 4 banks
            tc.tile_pool(name="u_ps", bufs=1, space="PSUM") as u_ps,      # 2 banks
            tc.tile_pool(name="pp_ps", bufs=2, space="PSUM") as pp_ps,    # 2 banks
        ):
            # ---- resident SBUF ----
            qTp = res.tile([128, S], BF16, tag="qTp")     # heads 0,1
            qTs = res.tile([64, S], BF16, tag="qTs")      # head 2
            kTp = res.tile([128, sk], BF16, tag="kTp")
            kTs = res.tile([64, sk], BF16, tag="kTs")
            v_aug = res.tile([128, kb * NH * 65], BF16, tag="vaug")
            woA = res.tile([128, D], BF16, tag="woA")
            woB = res.tile([64, D], BF16, tag="woB")
            smt = res.tile([128, kb + 4], F32, tag="smt")
            mbt = smt[:, 0:kb]
            bqkA = smt[:, kb:kb + 2]
            bqkB = smt[0:64, kb + 2:kb + 4]
            wq_sb = res.tile([128, DCH * E], BF16, tag="wq")
            wk_sb = res.tile([128, DCH * E], BF16, tag="wk")
            wv_sb = res.tile([128, DCH * E], BF16, tag="wv")
            xk_all = res.tile([128, DCH * sk], BF16, tag="xka")
            xv_all = res.tile([128, DCH * sk], BF16, tag="xva")
            xq_all = res.tile([128, DCH * S], BF16, tag="xqa")
            xkch = [xk_all[:, dc * sk:(dc + 1) * sk] for dc in range(DCH)]
            xvch = [xv_all[:, dc * sk:(dc + 1) * sk] for dc in range(DCH)]
            xqch = [xq_all[:, dc * S:(dc + 1) * S] for dc in range(DCH)]
            xTA = [
                res.tile([128, QW], BF16, tag=f"xTA{j}", name=f"xTA{j}")
                for j in range(QC)
            ]
            xTB = [
                res.tile([64, QW], BF16, tag=f"xTB{j}", name=f"xTB{j}")
                for j in range(QC)
            ]

            # ---- DMAs (program order = queue order: in the order compute
            # consumes them - K proj first, wo last) ----
            # ones columns of v_aug via engine memset (a strided DMA here
            # costs thousands of tiny descriptors and stalls the queue)
            nc.vector.memset(
                v_aug[:].rearrange("p (g c) -> p g c", c=65)[:, :, 64:65], 1.0
            )
            nc.sync.dma_start(out=wk_sb[:], in_=wk[:, :])
            for dc in range(DCH):
                nc.sync.dma_start(
                    out=xk_all[:, dc * sk:(dc + 1) * sk],
                    in_=xk[dc * 128:(dc + 1) * 128, :],
                )
            nc.sync.dma_start(out=wv_sb[:], in_=wv[:, :])
            for dc in range(DCH):
                nc.sync.dma_start(
                    out=xv_all[:, dc * sk:(dc + 1) * sk],
                    in_=xv[dc * 128:(dc + 1) * 128, :],
                )
            nc.sync.dma_start(out=smt[:], in_=sm[:, :])
            nc.sync.dma_start(out=wq_sb[:], in_=wq[:, :])
            for j in range(QC):
                nc.sync.dma_start(
                    out=xq_all[:].rearrange("p (c s) -> p c s", c=DCH)[
                        :, :, j * QW:(j + 1) * QW
                    ],
                    in_=xq[:, :].rearrange("(c p) s -> p c s", p=128)[
                        :, :, j * QW:(j + 1) * QW
                    ],
                )
            nc.sync.dma_start(out=woA[:], in_=wo[0:128, :])
            nc.sync.dma_start(out=woB[:], in_=wo[128:192, :])

            # ---- projection building blocks ----
            def qk_group(which, sc, ec):
                """One [ew, 512] Q/K projection group into pp, bias-add out."""
                if which == "q":
                    w_sb, xch, pair, single, ds_, scols = wq_sb, xqch, qTp, qTs, 0, S
                else:
                    w_sb, xch, pair, single, ds_, scols = wk_sb, xkch, kTp, kTs, 1, sk
                ew = 128 if ec == 0 else 64
                sw = min(512, scols - sc)
                ps = pp_ps.tile([128, 512], F32, tag="pp")
                for dc in range(DCH):
                    nc.tensor.matmul(
                        ps[:ew, :sw],
                        w_sb[:, dc * E + ec:dc * E + ec + ew],
                        xch[dc][:, sc:sc + sw],
                        start=(dc == 0),
                        stop=(dc == DCH - 1),
                    )
                # bias-add on ScalarE: keeps the DVE free for the softmax
                # normalize chain that these groups overlap with
                ident = mybir.ActivationFunctionType.Identity
                if ec == 0:
                    nc.scalar.activation(
                        pair[:, sc:sc + sw], ps[:128, :sw], ident,
                        bias=bqkA[:, ds_:ds_ + 1],
                    )
                else:
                    nc.scalar.activation(
                        single[:, sc:sc + sw], ps[:64, :sw], ident,
                        bias=bqkB[:, ds_:ds_ + 1],
                    )

            def v_group(b_):
                """V projection for key block b_ into v_aug (strided copy)."""
                vps = big.tile([128, QW], F32, tag="big", name=f"vps{b_}")
                for dc in range(DCH):
                    nc.tensor.matmul(
                        vps[:, 0:E],
                        xvch[dc][:, b_ * 128:(b_ + 1) * 128],
                        wv_sb[:, dc * E:(dc + 1) * E],
                        start=(dc == 0),
                        stop=(dc == DCH - 1),
                    )
                dst = v_aug[:, b_ * NH * 65:(b_ + 1) * NH * 65]
                nc.vector.tensor_copy(
                    dst.rearrange("p (g c) -> p g c", c=65)[:, :, 0:64],
                    vps[:, 0:NH * 64].rearrange("p (g c) -> p g c", c=64),
                )

            def out_group(qb, use_pp=False):
                """Output projection for q rows [qb*128, +128).

                use_pp alternates the PSUM source between the big pool and
                the pp pool so back-to-back tail groups pipeline 4 deep
                instead of 2."""
                jq, cq = qb // (QW // 128), (qb % (QW // 128)) * 128
                if use_pp:
                    t0 = pp_ps.tile([128, 512], F32, tag="pp", name=f"opp{qb}a")
                    t1 = pp_ps.tile([128, 512], F32, tag="pp", name=f"opp{qb}b")
                    parts = ((t0[:, 0:512], 0, 512), (t1[:, 0:256], 512, 256))
                else:
                    ops = big.tile([128, QW], F32, tag="big", name=f"ops{qb}")
                    parts = ((ops[:, 0:512], 0, 512), (ops[:, 512:768], 512, 256))
                for t, e0, ew in parts:
                    nc.tensor.matmul(
                        t, xTA[jq][:, cq:cq + 128], woA[:, e0:e0 + ew],
                        start=True, stop=False,
                    )
                    nc.tensor.matmul(
                        t, xTB[jq][:, cq:cq + 128], woB[:, e0:e0 + ew],
                        start=False, stop=True,
                    )
                ot = ocp.tile([128, D], BF16, tag="ot", name=f"ot{qb}")
                # split the PSUM->SBUF copy across ScalarE and DVE so it
                # drains in ~half the time
                nc.scalar.copy(ot[:, 0:512], parts[0][0])
                nc.vector.tensor_copy(ot[:, 512:768], parts[1][0])
                nc.sync.dma_start(out=out[qb * 128:(qb + 1) * 128, :], in_=ot[:, :])

            # ---- head phase: strictly in DMA-arrival order (the PE queue is
            # in-order, so a group whose inputs haven't landed would block
            # every later group) ----
            for sc in range(0, sk, 512):
                for ec in (0, 128):
                    qk_group("k", sc, ec)
            for ec in (0, 128):
                qk_group("q", 0, ec)
            for ec in (0, 128):
                qk_group("q", 512, ec)

            # ---- attention units with interleaved filler groups ----
            def att_unit(j, h, fillers, pre=None):
                if h < 2:
                    k_l = kTp[h * 64:(h + 1) * 64, :]
                    q_l = qTp[h * 64:(h + 1) * 64, :]
                else:
                    k_l = kTs[:, :]
                    q_l = qTs[:, :]
                u = u_ps.tile([65, QW], F32, tag="u")
                for b_ in range(kb):
                    if pre is not None and b_ < len(pre):
                        pre[b_]()
                    st = big.tile([128, QW], F32, tag="big", name=f"st{j}_{h}_{b_}")
                    for half in (0, 1):
                        nc.tensor.matmul(
                            st[:, half * 512:(half + 1) * 512],
                            k_l[:, b_ * 128:(b_ + 1) * 128],
                            q_l[:, j * QW + half * 512:j * QW + (half + 1) * 512],
                            start=True,
                            stop=True,
                        )
                    et = etp.tile([128, QW], BF16, tag="et")
                    nc.scalar.activation(
                        et[:, :], st[:, :], exp_f,
                        bias=mbt[:, b_:b_ + 1], scale=0.125,
                    )
                    vsl = v_aug[:, (b_ * NH + h) * 65:(b_ * NH + h) * 65 + 65]
                    for half in (0, 1):
                        nc.tensor.matmul(
                            u[:, half * 512:(half + 1) * 512],
                            vsl,
                            et[:, half * 512:(half + 1) * 512],
                            start=(b_ == 0),
                            stop=(b_ == kb - 1),
                        )
                # Drain u (numerators + denominator row) to SBUF in one
                # ScalarE copy: frees the u PSUM banks for the next unit's PV
                # matmuls after ~1us, and keeps the DVE counter untouched so
                # the filler matmuls' coarsened LDWEIGHTS waits (which
                # snapshot the DVE progress counter at emission) resolve
                # instantly.
                den = small.tile([1, QW], F32, tag="den")
                nc.scalar.copy(den[:, :], u[64:65, :])
                # PE filler work goes ahead of the DVE/GpSimd normalize chain
                # in every engine queue.
                for f in fillers:
                    f()
                # numerator drain on DVE (emitted after the fillers so their
                # coarsened LDWEIGHTS waits don't snapshot it)
                uc = small.tile([64, QW], F32, tag="uc")
                nc.vector.tensor_copy(uc[:, :], u[0:64, :])
                rec = small.tile([1, QW], F32, tag="rec")
                nc.vector.reciprocal_approx_fast(out=rec[:, :], in_=den[:, :])
                bcs = small.tile([64, QW], F32, tag="bcs")
                nc.gpsimd.partition_broadcast(bcs[:, :], rec[0:1, :])
                xdst = xTA[j][h * 64:(h + 1) * 64, :] if h < 2 else xTB[j][:, :]
                nc.vector.tensor_mul(xdst[:, :], uc[:, :], bcs[:, :])

            # j=0: V proj streams into unit (0,0); remaining Q chunks spread
            # across the three att(0) boundaries; j=1: out-proj as fillers
            att_unit(0, 0, [lambda: qk_group("q", 1024, 0)],
                     pre=[(lambda b=b_: v_group(b)) for b_ in range(kb)])
            att_unit(0, 1, [lambda: qk_group("q", 1536, 0),
                            lambda: qk_group("q", 1024, 128)])
            att_unit(0, 2, [lambda: qk_group("q", 1536, 128)])
            att_unit(1, 0, [lambda: out_group(0), lambda: out_group(1),
                            lambda: out_group(2)])
            att_unit(1, 1, [lambda: out_group(3), lambda: out_group(4),
                            lambda: out_group(5)])
            # out_group(6,7) target j=0, so they cover att(1,2)'s normalize
            # chain as fillers emitted ahead of it
            att_unit(1, 2, [lambda: out_group(6), lambda: out_group(7)])
            for qb in range(8, S // 128):
                out_group(qb, use_pp=(qb % 2 == 1))

    nc.compile()
    return nc


_PROGRAM_CACHE: dict[int, object] = {}


def _get_program(kb: int):
    if kb not in _PROGRAM_CACHE:
        _PROGRAM_CACHE[kb] = _build_program(kb)
    return _PROGRAM_CACHE[kb]


def _pack_w(w_t: np.ndarray, cols: int) -> np.ndarray:
    """[D, cols] weight -> [128, DCH*cols] SBUF-layout (chunks side by side)."""
    return np.ascontiguousarray(
        w_t.reshape(DCH, 128, cols).transpose(1, 0, 2).reshape(128, DCH * cols)
    )


def _prep_inputs(query, key, value, mask, Wq, bq, Wk, bk, Wv, bv, Wo, bo):
    """Host-side shard prep. Returns (in_maps, kb)."""
    f32 = np.float32
    valid = [np.nonzero(mask[b, 0, 0, :] != 0)[0] for b in range(B)]
    s_valid = max((len(v) for v in valid), default=1)
    s_pad = max(128, -(-s_valid // 128) * 128)
    kb = s_pad // 128

    per_batch = []
    for b in range(B):
        vi = valid[b]
        xq_t = np.ascontiguousarray(query[b].T).astype(BF_NP)
        xk_c = np.zeros((s_pad, D), dtype=f32)
        xv_c = np.zeros((s_pad, D), dtype=f32)
        xk_c[: len(vi)] = key[b][vi]
        xv_c[: len(vi)] = value[b][vi]
        mbias = np.full(s_pad, NEG, dtype=f32)
        mbias[: len(vi)] = 0.0
        per_batch.append(
            dict(
                xq_t=xq_t,
                xk_t=np.ascontiguousarray(xk_c.T).astype(BF_NP),
                xv_t=np.ascontiguousarray(xv_c.T).astype(BF_NP),
            )
        )
        per_batch[-1]["_mb"] = np.ascontiguousarray(mbias.reshape(kb, 128).T)

    in_maps = []
    for c in range(N_CORES):
        b = c // 4
        h0 = NH * (c % 4)
        sl = slice(h0 * DK, (h0 + NH) * DK)
        bqk_ = np.stack([bq[sl], bk[sl]], axis=1).astype(f32)
        smalls = np.zeros((128, kb + 4), dtype=f32)
        smalls[:, 0:kb] = per_batch[b]["_mb"]
        smalls[:, kb:kb + 2] = bqk_[0:128]
        smalls[0:64, kb + 2:kb + 4] = bqk_[128:192]
        pb = {k: v for k, v in per_batch[b].items() if k != "_mb"}
        in_maps.append(
            dict(
                pb,
                wq_t=_pack_w(Wq[sl, :].T.astype(BF_NP), E),
                wk_t=_pack_w(Wk[sl, :].T.astype(BF_NP), E),
                wv_t=_pack_w(Wv[sl, :].T.astype(BF_NP), E),
                wo_t=np.ascontiguousarray(Wo[:, sl].T).astype(BF_NP),
                smalls=smalls,
            )
        )
    return in_maps, kb


def kernel(query, key, value, mask, Wq, bq, Wk, bk, Wv, bv, Wo, bo):
    from concourse.bass_utils import run_bass_kernel_spmd

    query = np.asarray(query, dtype=np.float32)
    key = np.asarray(key, dtype=np.float32)
    value = np.asarray(value, dtype=np.float32)
    mask = np.asarray(mask)
    Wq, Wk, Wv, Wo = (np.asarray(a, dtype=np.float32) for a in (Wq, Wk, Wv, Wo))
    bq, bk, bv, bo = (np.asarray(a, dtype=np.float32) for a in (bq, bk, bv, bo))

    in_maps, kb = _prep_inputs(
        query, key, value, mask, Wq, bq, Wk, bk, Wv, bv, Wo, bo
    )
    nc = _get_program(kb)
    res = run_bass_kernel_spmd(nc, in_maps, core_ids=list(range(N_CORES)))

    out = np.zeros((B, S, D), dtype=np.float32)
    for c in range(N_CORES):
        out[c // 4] += res.results[c]["out"].astype(np.float32)
    # bv folds into the output as (sum_k p == 1) -> + bv @ Wo.T; bo is a plain
    # output bias. Both are zero for this problem's inputs; keep exactness for
    # any input without on-device cost.
    if np.any(bv) or np.any(bo):
        out += (bv @ Wo.T + bo)[None, None, :]
    return out


# revision 37
# speedup vs baseline: 1.1476x; 1.1476x over previous
"""Multi-headed attention (B=2, S=2048, D=768, H=12) on 8 TRN2 NeuronCores.

Sharding: data parallel on batch x tensor parallel on heads. Core c handles
batch c//4 and heads 3*(c%4) .. 3*(c%4)+2. Each core computes its partial
output projection [S, D]; the host sums the 4 partials per batch.

Key-position compaction: the mask is per key position only ([B,1,1,S],
values 0/1). Masked keys contribute exp(-1e9) == 0.0 exactly (fp32
underflow) to every softmax row, so the host drops masked key/value
positions before projection and pads to a multiple of 128; padded rows get
a -1e9 additive bias on the scores (same underflow-to-zero as the
reference's where(mask==0, -1e9, scores)). This is exact, not approximate.

Softmax runs without max-subtraction: scores ~ N(0,1) after the 1/sqrt(dk)
scale, so exp() cannot overflow; the reference's max-subtraction only
shifts numerator and denominator by a common factor.

Differences vs the first working version (283 us -> ~160 us):
  * bf16 operands everywhere on the matmul path (halves HBM traffic and
    SBUF footprint; PE rate is 1 cycle/row for bf16 same as fp32r).
  * scores tiles are [128, 1024] PSUM pairs (two banks, one matmul per
    bank) so each ScalarE exp instruction covers 1024 columns - halves
    the per-instruction overhead on the engine that paces attention.
  * Q-projection and output-projection matmul groups are interleaved
    between attention (j,h) units so the in-order PE queue always holds
    independent work: the HAM clock gate re-throttles the PE to 1.2 GHz
    whenever it sees idle gaps, which is where most of the baseline's
    time went (231 us of 291 us at K=4/8). A thermal/power duty cap
    still forces ~50% K=4/8 after ~45 us of sustained matmul activity;
    run-to-run variance from chip temperature is ~+/-15 us.
  * softmax normalization: u (PV output, with the ones-row denominator)
    drains to SBUF immediately (denominator row via ScalarE so the
    reciprocal can start; numerators via DVE) which frees the u PSUM
    bank for the next unit ~4 us earlier; 1/denom uses the single-op
    reciprocal_approx_fast (the exact InstReciprocal costs ~6.5 us per
    [1,1024] row); the column broadcast runs on the otherwise-idle
    GpSimd engine (partition_broadcast), not a PE matmul.
    NOTE: reciprocal_approx_fast silently produces garbage when its
    input AP is in PSUM or at a nonzero partition offset - it must read
    a [1, N] SBUF tile at partition 0.
  * DMAs are ordered strictly in consumption order; the v_aug ones
    columns come from an engine memset (the strided DMA cost thousands
    of descriptors); mask-bias and biases ride in one merged 'smalls'
    tensor (each extra DMA costs ~620 ns of issue time on the queue).

On-device layouts (per core):
  qT [e_local, s]   e_local = 3 local heads x 64 = 192, stored as a
                    [128, 2048] pair tile (heads 0,1) + [64, 2048] tile
  kT [e_local, kpos] same split, kpos compacted+padded to S_pad
  v_aug [128, KB*3*65] - per (kblock, head): 64 v columns + a ones column
                    (the ones column makes the PV matmul also produce the
                    softmax denominator as row 64 of the PSUM tile)
  scores are computed transposed, sT[kpos, q], so the pad-bias is a
  per-partition scalar and exp() needs a single ScalarE pass per tile.
"""

import sys

for _p in ("/opt/trn_rl_repo",):
    if _p not in sys.path:
        sys.path.insert(0, _p)

import numpy as np
import ml_dtypes

import concourse.bacc as bacc
import concourse.mybir as mybir
import concourse.tile as tile

B, S, D, H = 2, 2048, 768, 12
DK = D // H          # 64
NH = 3               # heads per core
E = NH * DK          # 192 local e width
N_CORES = 8
QW = 1024            # attention q tile (two PSUM banks)
QC = S // QW         # 2
DCH = D // 128       # 6 contraction chunks for the projections
NEG = -1.0e9

F32 = mybir.dt.float32
F32R = mybir.dt.float32r
BF16 = mybir.dt.bfloat16
BF_NP = ml_dtypes.bfloat16


def _build_program(kb: int):
    """Build the single-core SPMD program for KB key blocks of 128."""
    sk = kb * 128
    nc = bacc.Bacc("TRN2", target_bir_lowering=False, debug=False)

    xq = nc.dram_tensor("xq_t", [D, S], BF16, kind="ExternalInput").ap()
    xk = nc.dram_tensor("xk_t", [D, sk], BF16, kind="ExternalInput").ap()
    xv = nc.dram_tensor("xv_t", [D, sk], BF16, kind="ExternalInput").ap()
    wq = nc.dram_tensor("wq_t", [128, DCH * E], BF16, kind="ExternalInput").ap()
    wk = nc.dram_tensor("wk_t", [128, DCH * E], BF16, kind="ExternalInput").ap()
    wv = nc.dram_tensor("wv_t", [128, DCH * E], BF16, kind="ExternalInput").ap()
    wo = nc.dram_tensor("wo_t", [E, D], BF16, kind="ExternalInput").ap()
    sm = nc.dram_tensor("smalls", [128, kb + 4], F32, kind="ExternalInput").ap()
    out = nc.dram_tensor("out", [S, D], BF16, kind="ExternalOutput").ap()

    exp_f = mybir.ActivationFunctionType.Exp

    with tile.TileContext(nc) as tc:
        with (
            tc.tile_pool(name="resident", bufs=1) as res,
            tc.tile_pool(name="eT", bufs=4) as etp,
            tc.tile_pool(name="small", bufs=2) as small,
            tc.tile_pool(name="ocp", bufs=6) as ocp,
            tc.tile_pool(name="big_ps", bufs=2, space="PSUM") as big,     # 4 banks
            tc.tile_pool(name="u_ps", bufs=1, space="PSUM") as u_ps,      # 2 banks
            tc.tile_pool(name="pp_ps", bufs=2, space="PSUM") as pp_ps,    # 2 banks
        ):
            # ---- resident SBUF ----
            qTp = res.tile([128, S], BF16, tag="qTp")     # heads 0,1
            qTs = res.tile([64, S], BF16, tag="qTs")      # head 2
            kTp = res.tile([128, sk], BF16, tag="kTp")
            kTs = res.tile([64, sk], BF16, tag="kTs")
            v_aug = res.tile([128, kb * NH * 65], BF16, tag="vaug")
            woA = res.tile([128, D], BF16, tag="woA")
            woB = res.tile([64, D], BF16, tag="woB")
            smt = res.tile([128, kb + 4], F32, tag="smt")
            mbt = smt[:, 0:kb]
            bqkA = smt[:, kb:kb + 2]
            bqkB = smt[0:64, kb + 2:kb + 4]
            wq_sb = res.tile([128, DCH * E], BF16, tag="wq")
            wk_sb = res.tile([128, DCH * E], BF16, tag="wk")
            wv_sb = res.tile([128, DCH * E], BF16, tag="wv")
            xk_all = res.tile([128, DCH * sk], BF16, tag="xka")
            xv_all = res.tile([128, DCH * sk], BF16, tag="xva")
            xq_all = res.tile([128, DCH * S], BF16, tag="xqa")
            xkch = [xk_all[:, dc * sk:(dc + 1) * sk] for dc in range(DCH)]
            xvch = [xv_all[:, dc * sk:(dc + 1) * sk] for dc in range(DCH)]
            xqch = [xq_all[:, dc * S:(dc + 1) * S] for dc in range(DCH)]
            xTA = [
                res.tile([128, QW], BF16, tag=f"xTA{j}", name=f"xTA{j}")
                for j in range(QC)
            ]
            xTB = [
                res.tile([64, QW], BF16, tag=f"xTB{j}", name=f"xTB{j}")
                for j in range(QC)
            ]

            # ---- DMAs (program order = queue order: in the order compute
            # consumes them - K proj first, wo last) ----
            # ones columns of v_aug via engine memset (a strided DMA here
            # costs thousands of tiny descriptors and stalls the queue)
            nc.vector.memset(
                v_aug[:].rearrange("p (g c) -> p g c", c=65)[:, :, 64:65], 1.0
            )
            nc.sync.dma_start(out=wk_sb[:], in_=wk[:, :])
            for dc in range(DCH):
                nc.sync.dma_start(
                    out=xk_all[:, dc * sk:(dc + 1) * sk],
                    in_=xk[dc * 128:(dc + 1) * 128, :],
                )
            nc.sync.dma_start(out=wv_sb[:], in_=wv[:, :])
            for dc in range(DCH):
                nc.sync.dma_start(
                    out=xv_all[:, dc * sk:(dc + 1) * sk],
                    in_=xv[dc * 128:(dc + 1) * 128, :],
                )
            nc.sync.dma_start(out=smt[:], in_=sm[:, :])
            nc.sync.dma_start(out=wq_sb[:], in_=wq[:, :])
            for j in range(QC):
                nc.sync.dma_start(
                    out=xq_all[:].rearrange("p (c s) -> p c s", c=DCH)[
                        :, :, j * QW:(j + 1) * QW
                    ],
                    in_=xq[:, :].rearrange("(c p) s -> p c s", p=128)[
                        :, :, j * QW:(j + 1) * QW
                    ],
                )
            nc.sync.dma_start(out=woA[:], in_=wo[0:128, :])
            nc.sync.dma_start(out=woB[:], in_=wo[128:192, :])

            # ---- projection building blocks ----
            def qk_group(which, sc, ec):
                """One [ew, 512] Q/K projection group into pp, bias-add out."""
                if which == "q":
                    w_sb, xch, pair, single, ds_, scols = wq_sb, xqch, qTp, qTs, 0, S
                else:
                    w_sb, xch, pair, single, ds_, scols = wk_sb, xkch, kTp, kTs, 1, sk
                ew = 128 if ec == 0 else 64
                sw = min(512, scols - sc)
                ps = pp_ps.tile([128, 512], F32, tag="pp")
                for dc in range(DCH):
                    nc.tensor.matmul(
                        ps[:ew, :sw],
                        w_sb[:, dc * E + ec:dc * E + ec + ew],
                        xch[dc][:, sc:sc + sw],
                        start=(dc == 0),
                        stop=(dc == DCH - 1),
                    )
                # bias-add on ScalarE: keeps the DVE free for the softmax
                # normalize chain that these groups overlap with
                ident = mybir.ActivationFunctionType.Identity
                if ec == 0:
                    nc.scalar.activation(
                        pair[:, sc:sc + sw], ps[:128, :sw], ident,
                        bias=bqkA[:, ds_:ds_ + 1],
                    )
                else:
                    nc.scalar.activation(
                        single[:, sc:sc + sw], ps[:64, :sw], ident,
                        bias=bqkB[:, ds_:ds_ + 1],
                    )

            def v_group(b_):
                """V projection for key block b_ into v_aug (strided copy)."""
                vps = big.tile([128, QW], F32, tag="big", name=f"vps{b_}")
                for dc in range(DCH):
                    nc.tensor.matmul(
                        vps[:, 0:E],
                        xvch[dc][:, b_ * 128:(b_ + 1) * 128],
                        wv_sb[:, dc * E:(dc + 1) * E],
                        start=(dc == 0),
                        stop=(dc == DCH - 1),
                    )
                dst = v_aug[:, b_ * NH * 65:(b_ + 1) * NH * 65]
                nc.vector.tensor_copy(
                    dst.rearrange("p (g c) -> p g c", c=65)[:, :, 0:64],
                    vps[:, 0:NH * 64].rearrange("p (g c) -> p g c", c=64),
                )

            def out_group(qb, use_pp=False):
                """Output projection for q rows [qb*128, +128).

                use_pp alternates the PSUM source between the big pool and
                the pp pool so back-to-back tail groups pipeline 4 deep
                instead of 2."""
                jq, cq = qb // (QW // 128), (qb % (QW // 128)) * 128
                if use_pp:
                    t0 = pp_ps.tile([128, 512], F32, tag="pp", name=f"opp{qb}a")
                    t1 = pp_ps.tile([128, 512], F32, tag="pp", name=f"opp{qb}b")
                    parts = ((t0[:, 0:512], 0, 512), (t1[:, 0:256], 512, 256))
                else:
                    ops = big.tile([128, QW], F32, tag="big", name=f"ops{qb}")
                    parts = ((ops[:, 0:512], 0, 512), (ops[:, 512:768], 512, 256))
                for t, e0, ew in parts:
                    nc.tensor.matmul(
                        t, xTA[jq][:, cq:cq + 128], woA[:, e0:e0 + ew],
                        start=True, stop=False,
                    )
                    nc.tensor.matmul(
                        t, xTB[jq][:, cq:cq + 128], woB[:, e0:e0 + ew],
                        start=False, stop=True,
                    )
                ot = ocp.tile([128, D], BF16, tag="ot", name=f"ot{qb}")
                # split the PSUM->SBUF copy across ScalarE and DVE so it
                # drains in ~half the time
                nc.scalar.copy(ot[:, 0:512], parts[0][0])
                nc.vector.tensor_copy(ot[:, 512:768], parts[1][0])
                nc.sync.dma_start(out=out[qb * 128:(qb + 1) * 128, :], in_=ot[:, :])

            # ---- head phase: strictly in DMA-arrival order (the PE queue is
            # in-order, so a group whose inputs haven't landed would block
            # every later group) ----
            for sc in range(0, sk, 512):
                for ec in (0, 128):
                    qk_group("k", sc, ec)
            for b_ in range(kb):
                v_group(b_)
            for ec in (0, 128):
                qk_group("q", 0, ec)
            for ec in (0, 128):
                qk_group("q", 512, ec)

            # ---- attention units with interleaved filler groups ----
            def att_unit(j, h, fillers, pre=None):
                if h < 2:
                    k_l = kTp[h * 64:(h + 1) * 64, :]
                    q_l = qTp[h * 64:(h + 1) * 64, :]
                else:
                    k_l = kTs[:, :]
                    q_l = qTs[:, :]
                u = u_ps.tile([65, QW], F32, tag="u")
                for b_ in range(kb):
                    if pre is not None and b_ < len(pre):
                        pre[b_]()
                    st = big.tile([128, QW], F32, tag="big", name=f"st{j}_{h}_{b_}")
                    for half in (0, 1):
                        nc.tensor.matmul(
                            st[:, half * 512:(half + 1) * 512],
                            k_l[:, b_ * 128:(b_ + 1) * 128],
                            q_l[:, j * QW + half * 512:j * QW + (half + 1) * 512],
                            start=True,
                            stop=True,
                        )
                    et = etp.tile([128, QW], BF16, tag="et")
                    nc.scalar.activation(
                        et[:, :], st[:, :], exp_f,
                        bias=mbt[:, b_:b_ + 1], scale=0.125,
                    )
                    vsl = v_aug[:, (b_ * NH + h) * 65:(b_ * NH + h) * 65 + 65]
                    for half in (0, 1):
                        nc.tensor.matmul(
                            u[:, half * 512:(half + 1) * 512],
                            vsl,
                            et[:, half * 512:(half + 1) * 512],
                            start=(b_ == 0),
                            stop=(b_ == kb - 1),
                        )
                # Drain u (numerators + denominator row) to SBUF in one
                # ScalarE copy: frees the u PSUM banks for the next unit's PV
                # matmuls after ~1us, and keeps the DVE counter untouched so
                # the filler matmuls' coarsened LDWEIGHTS waits (which
                # snapshot the DVE progress counter at emission) resolve
                # instantly.
                den = small.tile([1, QW], F32, tag="den")
                nc.scalar.copy(den[:, :], u[64:65, :])
                # PE filler work goes ahead of the DVE/GpSimd normalize chain
                # in every engine queue.
                for f in fillers:
                    f()
                # numerator drain on DVE (emitted after the fillers so their
                # coarsened LDWEIGHTS waits don't snapshot it)
                uc = small.tile([64, QW], F32, tag="uc")
                nc.vector.tensor_copy(uc[:, :], u[0:64, :])
                rec = small.tile([1, QW], F32, tag="rec")
                nc.vector.reciprocal_approx_fast(out=rec[:, :], in_=den[:, :])
                bcs = small.tile([64, QW], F32, tag="bcs")
                nc.gpsimd.partition_broadcast(bcs[:, :], rec[0:1, :])
                xdst = xTA[j][h * 64:(h + 1) * 64, :] if h < 2 else xTB[j][:, :]
                nc.vector.tensor_mul(xdst[:, :], uc[:, :], bcs[:, :])

            # j=0: V proj streams into unit (0,0); remaining Q chunks spread
            # across the three att(0) boundaries; j=1: out-proj as fillers
            att_unit(0, 0, [lambda: qk_group("q", 1024, 0)])
            att_unit(0, 1, [lambda: qk_group("q", 1536, 0),
                            lambda: qk_group("q", 1024, 128)])
            att_unit(0, 2, [lambda: qk_group("q", 1536, 128)])
            att_unit(1, 0, [lambda: out_group(0), lambda: out_group(1),
                            lambda: out_group(2)])
            att_unit(1, 1, [lambda: out_group(3), lambda: out_group(4),
                            lambda: out_group(5)])
            # out_group(6,7) target j=0, so they cover att(1,2)'s normalize
            # chain as fillers emitted ahead of it
            att_unit(1, 2, [lambda: out_group(6), lambda: out_group(7)])
            for qb in range(8, S // 128):
                out_group(qb, use_pp=(qb % 2 == 1))

    nc.compile()
    return nc


_PROGRAM_CACHE: dict[int, object] = {}


def _get_program(kb: int):
    if kb not in _PROGRAM_CACHE:
        _PROGRAM_CACHE[kb] = _build_program(kb)
    return _PROGRAM_CACHE[kb]


def _pack_w(w_t: np.ndarray, cols: int) -> np.ndarray:
    """[D, cols] weight -> [128, DCH*cols] SBUF-layout (chunks side by side)."""
    return np.ascontiguousarray(
        w_t.reshape(DCH, 128, cols).transpose(1, 0, 2).reshape(128, DCH * cols)
    )


def _prep_inputs(query, key, value, mask, Wq, bq, Wk, bk, Wv, bv, Wo, bo):
    """Host-side shard prep. Returns (in_maps, kb)."""
    f32 = np.float32
    valid = [np.nonzero(mask[b, 0, 0, :] != 0)[0] for b in range(B)]
    s_valid = max((len(v) for v in valid), default=1)
    s_pad = max(128, -(-s_valid // 128) * 128)
    kb = s_pad // 128

    per_batch = []
    for b in range(B):
        vi = valid[b]
        xq_t = np.ascontiguousarray(query[b].T).astype(BF_NP)
        xk_c = np.zeros((s_pad, D), dtype=f32)
        xv_c = np.zeros((s_pad, D), dtype=f32)
        xk_c[: len(vi)] = key[b][vi]
        xv_c[: len(vi)] = value[b][vi]
        mbias = np.full(s_pad, NEG, dtype=f32)
        mbias[: len(vi)] = 0.0
        per_batch.append(
            dict(
                xq_t=xq_t,
                xk_t=np.ascontiguousarray(xk_c.T).astype(BF_NP),
                xv_t=np.ascontiguousarray(xv_c.T).astype(BF_NP),
            )
        )
        per_batch[-1]["_mb"] = np.ascontiguousarray(mbias.reshape(kb, 128).T)

    in_maps = []
    for c in range(N_CORES):
        b = c // 4
        h0 = NH * (c % 4)
        sl = slice(h0 * DK, (h0 + NH) * DK)
        bqk_ = np.stack([bq[sl], bk[sl]], axis=1).astype(f32)
        smalls = np.zeros((128, kb + 4), dtype=f32)
        smalls[:, 0:kb] = per_batch[b]["_mb"]
        smalls[:, kb:kb + 2] = bqk_[0:128]
        smalls[0:64, kb + 2:kb + 4] = bqk_[128:192]
        pb = {k: v for k, v in per_batch[b].items() if k != "_mb"}
        in_maps.append(
            dict(
                pb,
                wq_t=_pack_w(Wq[sl, :].T.astype(BF_NP), E),
                wk_t=_pack_w(Wk[sl, :].T.astype(BF_NP), E),
                wv_t=_pack_w(Wv[sl, :].T.astype(BF_NP), E),
                wo_t=np.ascontiguousarray(Wo[:, sl].T).astype(BF_NP),
                smalls=smalls,
            )
        )
    return in_maps, kb


def kernel(query, key, value, mask, Wq, bq, Wk, bk, Wv, bv, Wo, bo):
    from concourse.bass_utils import run_bass_kernel_spmd

    query = np.asarray(query, dtype=np.float32)
    key = np.asarray(key, dtype=np.float32)
    value = np.asarray(value, dtype=np.float32)
    mask = np.asarray(mask)
    Wq, Wk, Wv, Wo = (np.asarray(a, dtype=np.float32) for a in (Wq, Wk, Wv, Wo))
    bq, bk, bv, bo = (np.asarray(a, dtype=np.float32) for a in (bq, bk, bv, bo))

    in_maps, kb = _prep_inputs(
        query, key, value, mask, Wq, bq, Wk, bk, Wv, bv, Wo, bo
    )
    nc = _get_program(kb)
    res = run_bass_kernel_spmd(nc, in_maps, core_ids=list(range(N_CORES)))

    out = np.zeros((B, S, D), dtype=np.float32)
    for c in range(N_CORES):
        out[c // 4] += res.results[c]["out"].astype(np.float32)
    # bv folds into the output as (sum_k p == 1) -> + bv @ Wo.T; bo is a plain
    # output bias. Both are zero for this problem's inputs; keep exactness for
    # any input without on-device cost.
    if np.any(bv) or np.any(bo):
        out += (bv @ Wo.T + bo)[None, None, :]
    return out
